# revision 17
# baseline (speedup 1.0000x reference)
"""Local (windowed) attention kernel for Trainium2, sequence-parallel over 8 NeuronCores.

Reference computation (fp32):
    qkv = x @ w_qkv ; q,k,v split, reshaped to (head, window, 128, 64)
    k,v get a 1-window zero-padded lookback -> (head, window, 256, 64)
    sim = q @ k.T * d^-0.5, causal-banded mask, softmax, out = attn @ v
    y = out @ w_out + b_out

Sharding: 128 windows of 128 tokens -> 16 windows per core, plus a 128-row
halo of x from the previous core (zeros for core 0, which exactly reproduces
the reference's zero-pad lookback including its effect on the softmax
denominator). No inter-core communication.

Device dataflow (per core, bf16 matmuls accumulating in fp32):
  qkT = w_qk.T @ xT keeps head features on partitions; v = xT.T @ w_v keeps
  tokens on partitions with a ones-column per head so attn@v also emits the
  softmax denominator. Scores are computed transposed (pT[j, i]) and j-batched:
  one matmul per k-window covers both q-windows that see it, with the two
  heads of a pair row-tiled onto disjoint PE row groups. attn@v accumulates
  four output windows into one PSUM bank with shingled N=256 matmuls; the
  softmax division is denom-row -> K=1 broadcast matmul -> one fused
  reciprocal_approx_fast -> one multiply, all at [64, 512] granularity so no
  lane-starved [1, N] vector work remains. Softmax skips max-subtraction
  (logits are ~N(0, 0.4); exp is safe in fp32).
"""

import sys

sys.path.insert(0, "/opt/trn_rl_repo")

import numpy as np
import ml_dtypes

import concourse.bass as bass
import concourse.mybir as mybir
import concourse.tile as tile
from concourse import bacc
from concourse.bass_utils import run_bass_kernel_spmd

BF16 = mybir.dt.bfloat16
F32 = mybir.dt.float32

N = 16384
DIM = 1024
HEADS = 8
DHEAD = 64
WSZ = 128
NCORES = 8
R = N // NCORES            # 2048 own rows per core
T = R + WSZ                # 2176 rows incl. halo
NW = R // WSZ              # 16 own windows
DK = DIM // 128            # 8 contraction chunks
P = 128
SCALE = DHEAD ** -0.5

# token blocks for the qkT projection; q skips the halo window (its queries
# are never used), k covers all 17 windows
K_BLOCKS = [(0, 512), (512, 512), (1024, 512), (1536, 512), (2048, 128)]
Q_BLOCKS = [(128, 512), (640, 512), (1152, 512), (1664, 512)]

_CACHE = {}


def _build():
    nc = bacc.Bacc()
    xT_d = nc.declare_dram_parameter("xT", [DIM, T], BF16, isOutput=False)
    wqkv_d = nc.declare_dram_parameter("wqkv", [DIM, 3 * HEADS * DHEAD], BF16, isOutput=False)
    wout_d = nc.declare_dram_parameter("wout", [HEADS * DHEAD, DIM], BF16, isOutput=False)
    maskT_d = nc.declare_dram_parameter("maskT", [P, P], BF16, isOutput=False)
    ones1_d = nc.declare_dram_parameter("ones1", [1, DHEAD], BF16, isOutput=False)
    import os
    OUT_DT = F32 if os.environ.get("OUT_F32") else BF16
    out_d = nc.declare_dram_parameter("out", [R, DIM], OUT_DT, isOutput=True)

    with tile.TileContext(nc) as tc:
        with (
            tc.tile_pool(name="pers", bufs=1) as pers,
            tc.tile_pool(name="ptp", bufs=8) as ptp,
            tc.tile_pool(name="dro", bufs=4) as dro,
            tc.tile_pool(name="b4p", bufs=4) as b4p,
            tc.tile_pool(name="outp", bufs=3) as outp,
            tc.tile_pool(name="ps512", bufs=4, space="PSUM") as ps512,
            tc.tile_pool(name="pspt", bufs=1, space="PSUM") as pspt,
            tc.tile_pool(name="pso4", bufs=2, space="PSUM") as pso4,
        ):
            # ---- phase A: load inputs -------------------------------------
            # per-chunk DMAs, issue split across the two HWDGE rings (sync +
            # scalar sequencers) so the ~650ns DIRECT2D issue costs overlap;
            # chunk k of xT and w_qkv land together so the k-accumulation
            # loops in the projections start before the full tensors arrive
            maskT_sb = pers.tile([P, P], BF16, tag="maskT")
            ones1_sb = pers.tile([1, DHEAD], BF16, tag="ones1")
            xTall = pers.tile([P, DK, T], BF16, tag="xTall")
            wall = pers.tile([P, DK, 3 * HEADS * DHEAD], BF16, tag="wall")
            woall = pers.tile([P, 4, DIM], BF16, tag="woall")
            xT_dr = xT_d.rearrange("(k p) t -> p k t", p=P)
            wqkv_dr = wqkv_d.rearrange("(k p) c -> p k c", p=P)
            wout_dr = wout_d.rearrange("(m p) c -> p m c", p=P)
            # sync ring: w_qkv then the late half of xT; scalar ring: early
            # half of xT then w_out. HWDGE rings drain FIFO per engine, so
            # the tensors needed first really do complete first.
            XSPLIT = 1152
            # sync ring: w_qkv column blocks in compute order, then late xT;
            # scalar ring: early xT chunks, mask/ones, then w_out.
            # B(m) only needs w columns m*128:(m+1)*128 of every chunk, so
            # column-block loads (256KB) unlock full k-loops within ~5us.
            nc.sync.dma_start(wall[:, :, 0:128], wqkv_dr[:, :, 0:128])
            nc.scalar.dma_start(xTall[:, 0, 0:XSPLIT], xT_dr[:, 0, 0:XSPLIT])
            nc.sync.dma_start(wall[:, :, 512:640], wqkv_dr[:, :, 512:640])
            nc.scalar.dma_start(xTall[:, 1, 0:XSPLIT], xT_dr[:, 1, 0:XSPLIT])
            nc.sync.dma_start(wall[:, :, 1024:1536], wqkv_dr[:, :, 1024:1536])
            for k in range(2, DK):
                nc.scalar.dma_start(xTall[:, k, 0:XSPLIT], xT_dr[:, k, 0:XSPLIT])
                if k == 2:
                    nc.scalar.dma_start(maskT_sb[:], maskT_d[:])
                    nc.scalar.dma_start(ones1_sb[:], ones1_d[:])
            nc.sync.dma_start(wall[:, :, 128:512], wqkv_dr[:, :, 128:512])
            nc.sync.dma_start(wall[:, :, 640:1024], wqkv_dr[:, :, 640:1024])
            nc.sync.dma_start(xTall[:, :, XSPLIT:T], xT_dr[:, :, XSPLIT:T])
            for m in range(4):
                nc.scalar.dma_start(woall[:, m, :], wout_dr[:, m, :])

            qk_sb = [pers.tile([P, T], BF16, tag=f"qk{m}", name=f"qk{m}") for m in range(8)]
            v_sb = [pers.tile([P, HEADS, DHEAD + 1], BF16, tag=f"v{t}", name=f"v{t}") for t in range(NW + 1)]
            attn_sb = [pers.tile([P, R], BF16, tag=f"at{m}", name=f"at{m}") for m in range(4)]

            def proj_qk(m, blocks=None):
                """qk_sb[m] = w_qkv[:, m-chunk].T @ xT  ([128, T])"""
                if blocks is None:
                    blocks = Q_BLOCKS if m < 4 else K_BLOCKS
                for (b0, bw) in blocks:
                    pq = ps512.tile([P, 512], F32, tag="mm512", name="mm512")
                    for k in range(DK):
                        nc.tensor.matmul(
                            pq[:, :bw],
                            lhsT=wall[:, k, m * P:(m + 1) * P],
                            rhs=xTall[:, k, b0:b0 + bw],
                            start=(k == 0), stop=(k == DK - 1),
                        )
                    nc.vector.tensor_copy(qk_sb[m][:, b0:b0 + bw], pq[:, :bw])

            def proj_v(trange):
                """v_sb[t] = xT[:, t-tile].T @ w_v, plus a ones column per head."""
                for t in trange:
                    nc.vector.memset(v_sb[t][:, :, DHEAD:DHEAD + 1], 1.0)
                    pv = ps512.tile([P, 512], F32, tag="mm512", name="mm512")
                    for k in range(DK):
                        nc.tensor.matmul(
                            pv[:],
                            lhsT=xTall[:, k, t * P:(t + 1) * P],
                            rhs=wall[:, k, 1024:1536],
                            start=(k == 0), stop=(k == DK - 1),
                        )
                    nc.vector.tensor_copy(
                        v_sb[t][:, :, 0:DHEAD],
                        pv.rearrange("p (h d) -> p h d", h=HEADS),
                    )

            # the two heads of a pair are row-tiled onto disjoint PE row
            # groups, so their score matmuls execute CONCURRENTLY — each head
            # must drain into its own PSUM bank (same-bank concurrent drains
            # are a fatal HW collision). Two persistent double-slot banks.
            ppt_hh = [pspt.tile([P, 2, 256], F32, tag=f"pT{hh}", name=f"pT{hh}")
                      for hh in range(2)]

            def scores(pr, t, pt_tiles):
                """j-batched transposed scores for k-window t, heads 2pr/2pr+1.

                ppt[:, 0:128]   = k_t . q_t     (current window, causal mask)
                ppt[:, 128:256] = k_t . q_{t+1} (t is its lookback window)
                """
                mq, mk = pr, 4 + pr
                lo = 128 if t == 0 else 0
                hi = 128 if t == NW else 256
                slot = t % 2
                for hh in range(2):
                    off = hh * 64
                    nc.tensor.matmul(
                        ppt_hh[hh][:, slot, lo:hi],
                        lhsT=qk_sb[mk][off:off + 64, t * P:(t + 1) * P],
                        rhs=qk_sb[mq][off:off + 64, t * P + lo:t * P + hi],
                        start=True, stop=True,
                    )
                pt = ptp.tile([P, 2, 256], BF16, tag="pt", name="pt")
                for hh in range(2):
                    nc.scalar.activation(pt[:, hh, lo:hi], ppt_hh[hh][:, slot, lo:hi],
                                         mybir.ActivationFunctionType.Exp, scale=SCALE)
                    if t > 0:
                        nc.gpsimd.tensor_mul(pt[:, hh, 0:P], pt[:, hh, 0:P], maskT_sb[:])
                pt_tiles[t] = pt

            def epilogue(pr, w0, nwin, pt_tiles):
                """attn@v + softmax divide for windows w0..w0+nwin-1."""
                mq = pr
                nc_ = nwin * P
                for hh in range(2):
                    off = hh * 64
                    po4 = pso4.tile([DHEAD + 1, 512], F32, tag="po4", name="po4")
                    # window w = w0+i in cols i*128:(i+1)*128: lookback window
                    # contribution (v_w, ptT_w[:, 128:256]) then current-window
                    # (v_{w+1}, ptT_{w+1}[:, 0:128]); each matmul targets one
                    # uniform 128-col slice of the shared bank
                    for i in range(nwin):
                        t0 = w0 + i
                        nc.tensor.matmul(
                            po4[:, i * P:(i + 1) * P],
                            lhsT=v_sb[t0][:, 2 * pr + hh, :],
                            rhs=pt_tiles[t0][:, hh, 128:256],
                            start=True, stop=False,
                        )
                        nc.tensor.matmul(
                            po4[:, i * P:(i + 1) * P],
                            lhsT=v_sb[t0 + 1][:, 2 * pr + hh, :],
                            rhs=pt_tiles[t0 + 1][:, hh, 0:128],
                            start=False, stop=True,
                        )
                    # softmax denominators rode along in row 64 (ones column);
                    # broadcast them across partitions on the idle GpSimd
                    # engine (a K=1 PE matmul pays ~115ns un-overlapped drain)
                    d4 = dro.tile([1, 512], F32, tag="d4", name="d4")
                    nc.scalar.copy(d4[:, :nc_], po4[DHEAD:DHEAD + 1, :nc_])
                    b4x = b4p.tile([DHEAD, 512], F32, tag="b4x", name="b4x")
                    nc.gpsimd.partition_broadcast(b4x[:, :nc_], d4[:, :nc_])
                    b4 = b4p.tile([DHEAD, 512], F32, tag="b4", name="b4")
                    nc.vector.reciprocal_approx_fast(b4[:, :nc_], b4x[:, :nc_])
                    nc.vector.tensor_mul(
                        attn_sb[mq][off:off + 64, w0 * P:w0 * P + nc_],
                        po4[0:DHEAD, :nc_], b4[:, :nc_],
                    )

            def out_window(t):
                o_sb = outp.tile([P, DIM], OUT_DT, tag="o_sb", name="o_sb")
                for nf in range(2):
                    pf = ps512.tile([P, 512], F32, tag="mm512", name="mm512")
                    for m in range(4):
                        nc.tensor.matmul(
                            pf[:],
                            lhsT=attn_sb[m][:, t * P:(t + 1) * P],
                            rhs=woall[:, m, nf * 512:(nf + 1) * 512],
                            start=(m == 0), stop=(m == 3),
                        )
                    if nf == 0:
                        nc.vector.tensor_copy(o_sb[:, 0:512], pf[:])
                    else:
                        nc.scalar.copy(o_sb[:, 512:1024], pf[:])
                eng = nc.sync if t % 2 == 0 else nc.scalar
                eng.dma_start(out_d[t * P:(t + 1) * P, :], o_sb[:])

            def attention(pr, emit_out=False):
                pt_tiles = {}
                scores(pr, 0, pt_tiles)
                nwin = 2 if emit_out else 4
                for w0 in range(0, NW, nwin):
                    for t in range(w0 + 1, w0 + nwin + 1):
                        scores(pr, t, pt_tiles)
                    epilogue(pr, w0, nwin, pt_tiles)
                    if emit_out:
                        for t in range(w0, w0 + nwin):
                            out_window(t)

            # issue order: interleave projections with attention head-pairs so
            # the PE always has dense matmul work while ACT/DVE chew on the
            # softmax chain of the previous head-pair
            # early half (token cols < XSPLIT) unlocks as soon as the first
            # xT DMA lands; late half follows the second xT DMA
            proj_qk(0, [(128, 512), (640, 512)])
            proj_qk(4, [(0, 512), (512, 512)])
            proj_v(range(0, 9))
            proj_qk(0, [(1152, 512), (1664, 512)])
            proj_qk(4, [(1024, 512), (1536, 512), (2048, 128)])
            proj_v(range(9, NW + 1))
            attention(0)
            for pr in range(1, 3):
                proj_qk(pr)
                proj_qk(4 + pr)
                attention(pr)
            proj_qk(3)
            proj_qk(7)
            attention(3, emit_out=True)

    nc.compile()
    return nc


def _get_nc():
    if "nc" not in _CACHE:
        _CACHE["nc"] = _build()
    return _CACHE["nc"]


def kernel(x, w_qkv, w_out, b_out):
    x = np.asarray(x, dtype=np.float32)
    w_qkv_b = np.asarray(w_qkv, dtype=np.float32).astype(ml_dtypes.bfloat16)
    w_out_b = np.asarray(w_out, dtype=np.float32).astype(ml_dtypes.bfloat16)
    b_out = np.asarray(b_out, dtype=np.float32)

    # maskT[j, i] = 1 where j <= i  (transposed causal mask for current window)
    maskT = np.triu(np.ones((P, P), dtype=np.float32)).astype(ml_dtypes.bfloat16)
    ones1 = np.ones((1, DHEAD), dtype=ml_dtypes.bfloat16)

    x_pad = np.concatenate([np.zeros((WSZ, DIM), np.float32), x], axis=0)
    in_maps = []
    for c in range(NCORES):
        x_sh = x_pad[c * R:c * R + T]                       # (2176, 1024)
        xT = np.ascontiguousarray(x_sh.T).astype(ml_dtypes.bfloat16)
        in_maps.append({
            "xT": xT,
            "wqkv": w_qkv_b,
            "wout": w_out_b,
            "maskT": maskT,
            "ones1": ones1,
        })

    nc = _get_nc()
    res = run_bass_kernel_spmd(nc, in_maps, core_ids=list(range(NCORES)))
    out = np.concatenate(
        [np.asarray(res.results[c]["out"]).astype(np.float32) for c in range(NCORES)],
        axis=0,
    )
    return out + b_out[None, :]


# revision 18
# speedup vs baseline: 2.7163x; 2.7163x over previous
"""Local (windowed) attention kernel for Trainium2, sequence-parallel over 8 NeuronCores.

Reference computation (fp32):
    qkv = x @ w_qkv ; q,k,v split, reshaped to (head, window, 128, 64)
    k,v get a 1-window zero-padded lookback -> (head, window, 256, 64)
    sim = q @ k.T * d^-0.5, causal-banded mask, softmax, out = attn @ v
    y = out @ w_out + b_out

Sharding: 128 windows of 128 tokens -> 16 windows per core, plus a 128-row
halo of x from the previous core (zeros for core 0, which exactly reproduces
the reference's zero-pad lookback including its effect on the softmax
denominator). No inter-core communication.

Device dataflow (per core, bf16 matmuls accumulating in fp32):
  qkT = w_qk.T @ xT keeps head features on partitions; v = xT.T @ w_v keeps
  tokens on partitions with a ones-column per head so attn@v also emits the
  softmax denominator. Scores are computed transposed (pT[j, i]) and j-batched:
  one matmul per k-window covers both q-windows that see it, with the two
  heads of a pair row-tiled onto disjoint PE row groups. attn@v accumulates
  four output windows into one PSUM bank with shingled N=256 matmuls; the
  softmax division is denom-row -> K=1 broadcast matmul -> one fused
  reciprocal_approx_fast -> one multiply, all at [64, 512] granularity so no
  lane-starved [1, N] vector work remains. Softmax skips max-subtraction
  (logits are ~N(0, 0.4); exp is safe in fp32).
"""

import sys

sys.path.insert(0, "/opt/trn_rl_repo")

import numpy as np
import ml_dtypes

import concourse.bass as bass
import concourse.mybir as mybir
import concourse.tile as tile
from concourse import bacc
from concourse.bass_utils import run_bass_kernel_spmd

BF16 = mybir.dt.bfloat16
F32 = mybir.dt.float32

N = 16384
DIM = 1024
HEADS = 8
DHEAD = 64
WSZ = 128
NCORES = 8
R = N // NCORES            # 2048 own rows per core
T = R + WSZ                # 2176 rows incl. halo
NW = R // WSZ              # 16 own windows
DK = DIM // 128            # 8 contraction chunks
P = 128
SCALE = DHEAD ** -0.5

# token blocks for the qkT projection; q skips the halo window (its queries
# are never used), k covers all 17 windows
K_BLOCKS = [(0, 512), (512, 512), (1024, 512), (1536, 512), (2048, 128)]
Q_BLOCKS = [(128, 512), (640, 512), (1152, 512), (1664, 512)]

_CACHE = {}


def _build():
    nc = bacc.Bacc()
    xT_d = nc.declare_dram_parameter("xT", [DIM, T], BF16, isOutput=False)
    wqkv_d = nc.declare_dram_parameter("wqkv", [DIM, 3 * HEADS * DHEAD], BF16, isOutput=False)
    wout_d = nc.declare_dram_parameter("wout", [HEADS * DHEAD, DIM], BF16, isOutput=False)
    maskT_d = nc.declare_dram_parameter("maskT", [P, P], BF16, isOutput=False)
    ones1_d = nc.declare_dram_parameter("ones1", [1, DHEAD], BF16, isOutput=False)
    import os
    OUT_DT = F32 if os.environ.get("OUT_F32") else BF16
    out_d = nc.declare_dram_parameter("out", [R, DIM], OUT_DT, isOutput=True)

    with tile.TileContext(nc) as tc:
        with (
            tc.tile_pool(name="pers", bufs=1) as pers,
            tc.tile_pool(name="ptp", bufs=8) as ptp,
            tc.tile_pool(name="dro", bufs=4) as dro,
            tc.tile_pool(name="b4p", bufs=4) as b4p,
            tc.tile_pool(name="outp", bufs=3) as outp,
            tc.tile_pool(name="ps512", bufs=4, space="PSUM") as ps512,
            tc.tile_pool(name="pspt", bufs=1, space="PSUM") as pspt,
            tc.tile_pool(name="pso4", bufs=2, space="PSUM") as pso4,
        ):
            # ---- phase A: load inputs -------------------------------------
            # per-chunk DMAs, issue split across the two HWDGE rings (sync +
            # scalar sequencers) so the ~650ns DIRECT2D issue costs overlap;
            # chunk k of xT and w_qkv land together so the k-accumulation
            # loops in the projections start before the full tensors arrive
            maskT_sb = pers.tile([P, P], BF16, tag="maskT")
            ones1_sb = pers.tile([1, DHEAD], BF16, tag="ones1")
            xTall = pers.tile([P, DK, T], BF16, tag="xTall")
            wall = pers.tile([P, DK, 3 * HEADS * DHEAD], BF16, tag="wall")
            woall = pers.tile([P, 4, DIM], BF16, tag="woall")
            xT_dr = xT_d.rearrange("(k p) t -> p k t", p=P)
            wqkv_dr = wqkv_d.rearrange("(k p) c -> p k c", p=P)
            wout_dr = wout_d.rearrange("(m p) c -> p m c", p=P)
            # sync ring: w_qkv then the late half of xT; scalar ring: early
            # half of xT then w_out. HWDGE rings drain FIFO per engine, so
            # the tensors needed first really do complete first.
            XSPLIT = 1152
            # sync ring: w_qkv column blocks in compute order, then late xT;
            # scalar ring: early xT chunks, mask/ones, then w_out.
            # B(m) only needs w columns m*128:(m+1)*128 of every chunk, so
            # column-block loads (256KB) unlock full k-loops within ~5us.
            nc.sync.dma_start(wall[:, :, 0:128], wqkv_dr[:, :, 0:128])
            nc.scalar.dma_start(xTall[:, 0, 0:XSPLIT], xT_dr[:, 0, 0:XSPLIT])
            nc.sync.dma_start(wall[:, :, 512:640], wqkv_dr[:, :, 512:640])
            nc.scalar.dma_start(xTall[:, 1, 0:XSPLIT], xT_dr[:, 1, 0:XSPLIT])
            nc.sync.dma_start(wall[:, :, 1024:1536], wqkv_dr[:, :, 1024:1536])
            for k in range(2, DK):
                nc.scalar.dma_start(xTall[:, k, 0:XSPLIT], xT_dr[:, k, 0:XSPLIT])
                if k == 2:
                    nc.scalar.dma_start(maskT_sb[:], maskT_d[:])
                    nc.scalar.dma_start(ones1_sb[:], ones1_d[:])
            nc.sync.dma_start(wall[:, :, 128:512], wqkv_dr[:, :, 128:512])
            nc.sync.dma_start(wall[:, :, 640:1024], wqkv_dr[:, :, 640:1024])
            nc.sync.dma_start(xTall[:, :, XSPLIT:T], xT_dr[:, :, XSPLIT:T])
            for m in range(4):
                nc.scalar.dma_start(woall[:, m, :], wout_dr[:, m, :])

            qk_sb = [pers.tile([P, T], BF16, tag=f"qk{m}", name=f"qk{m}") for m in range(8)]
            v_sb = [pers.tile([P, HEADS, DHEAD + 1], BF16, tag=f"v{t}", name=f"v{t}") for t in range(NW + 1)]
            attn_sb = [pers.tile([P, R], BF16, tag=f"at{m}", name=f"at{m}") for m in range(4)]

            def proj_qk(m, blocks=None):
                """qk_sb[m] = w_qkv[:, m-chunk].T @ xT  ([128, T])"""
                if blocks is None:
                    blocks = Q_BLOCKS if m < 4 else K_BLOCKS
                for (b0, bw) in blocks:
                    pq = ps512.tile([P, 512], F32, tag="mm512", name="mm512")
                    for k in range(DK):
                        nc.tensor.matmul(
                            pq[:, :bw],
                            lhsT=wall[:, k, m * P:(m + 1) * P],
                            rhs=xTall[:, k, b0:b0 + bw],
                            start=(k == 0), stop=(k == DK - 1),
                        )
                    nc.vector.tensor_copy(qk_sb[m][:, b0:b0 + bw], pq[:, :bw])

            def proj_v(trange):
                """v_sb[t] = xT[:, t-tile].T @ w_v, plus a ones column per head."""
                for t in trange:
                    nc.vector.memset(v_sb[t][:, :, DHEAD:DHEAD + 1], 1.0)
                    pv = ps512.tile([P, 512], F32, tag="mm512", name="mm512")
                    for k in range(DK):
                        nc.tensor.matmul(
                            pv[:],
                            lhsT=xTall[:, k, t * P:(t + 1) * P],
                            rhs=wall[:, k, 1024:1536],
                            start=(k == 0), stop=(k == DK - 1),
                        )
                    nc.vector.tensor_copy(
                        v_sb[t][:, :, 0:DHEAD],
                        pv.rearrange("p (h d) -> p h d", h=HEADS),
                    )

            # the two heads of a pair are row-tiled onto disjoint PE row
            # groups, so their score matmuls execute CONCURRENTLY — each head
            # must drain into its own PSUM bank (same-bank concurrent drains
            # are a fatal HW collision). Two persistent double-slot banks.
            ppt_hh = [pspt.tile([P, 2, 256], F32, tag=f"pT{hh}", name=f"pT{hh}")
                      for hh in range(2)]

            def scores(pr, t, pt_tiles):
                """j-batched transposed scores for k-window t, heads 2pr/2pr+1.

                ppt[:, 0:128]   = k_t . q_t     (current window, causal mask)
                ppt[:, 128:256] = k_t . q_{t+1} (t is its lookback window)
                """
                mq, mk = pr, 4 + pr
                lo = 128 if t == 0 else 0
                hi = 128 if t == NW else 256
                slot = t % 2
                for hh in range(2):
                    off = hh * 64
                    nc.tensor.matmul(
                        ppt_hh[hh][:, slot, lo:hi],
                        lhsT=qk_sb[mk][off:off + 64, t * P:(t + 1) * P],
                        rhs=qk_sb[mq][off:off + 64, t * P + lo:t * P + hi],
                        start=True, stop=True,
                    )
                pt = ptp.tile([P, 2, 256], BF16, tag="pt", name="pt")
                for hh in range(2):
                    nc.scalar.activation(pt[:, hh, lo:hi], ppt_hh[hh][:, slot, lo:hi],
                                         mybir.ActivationFunctionType.Exp, scale=SCALE)
                    if t > 0:
                        nc.vector.tensor_mul(pt[:, hh, 0:P], pt[:, hh, 0:P], maskT_sb[:])
                pt_tiles[t] = pt

            def epilogue(pr, w0, nwin, pt_tiles):
                """attn@v + softmax divide for windows w0..w0+nwin-1."""
                mq = pr
                nc_ = nwin * P
                for hh in range(2):
                    off = hh * 64
                    po4 = pso4.tile([DHEAD + 1, 512], F32, tag="po4", name="po4")
                    # window w = w0+i in cols i*128:(i+1)*128: lookback window
                    # contribution (v_w, ptT_w[:, 128:256]) then current-window
                    # (v_{w+1}, ptT_{w+1}[:, 0:128]); each matmul targets one
                    # uniform 128-col slice of the shared bank
                    for i in range(nwin):
                        t0 = w0 + i
                        nc.tensor.matmul(
                            po4[:, i * P:(i + 1) * P],
                            lhsT=v_sb[t0][:, 2 * pr + hh, :],
                            rhs=pt_tiles[t0][:, hh, 128:256],
                            start=True, stop=False,
                        )
                        nc.tensor.matmul(
                            po4[:, i * P:(i + 1) * P],
                            lhsT=v_sb[t0 + 1][:, 2 * pr + hh, :],
                            rhs=pt_tiles[t0 + 1][:, hh, 0:128],
                            start=False, stop=True,
                        )
                    # softmax denominators rode along in row 64 (ones column);
                    # broadcast them across partitions on the idle GpSimd
                    # engine (a K=1 PE matmul pays ~115ns un-overlapped drain)
                    d4 = dro.tile([1, 512], F32, tag="d4", name="d4")
                    nc.scalar.copy(d4[:, :nc_], po4[DHEAD:DHEAD + 1, :nc_])
                    b4x = b4p.tile([DHEAD, 512], F32, tag="b4x", name="b4x")
                    nc.gpsimd.partition_broadcast(b4x[:, :nc_], d4[:, :nc_])
                    b4 = b4p.tile([DHEAD, 512], F32, tag="b4", name="b4")
                    nc.vector.reciprocal_approx_fast(b4[:, :nc_], b4x[:, :nc_])
                    nc.vector.tensor_mul(
                        attn_sb[mq][off:off + 64, w0 * P:w0 * P + nc_],
                        po4[0:DHEAD, :nc_], b4[:, :nc_],
                    )

            def out_window(t):
                o_sb = outp.tile([P, DIM], OUT_DT, tag="o_sb", name="o_sb")
                for nf in range(2):
                    pf = ps512.tile([P, 512], F32, tag="mm512", name="mm512")
                    for m in range(4):
                        nc.tensor.matmul(
                            pf[:],
                            lhsT=attn_sb[m][:, t * P:(t + 1) * P],
                            rhs=woall[:, m, nf * 512:(nf + 1) * 512],
                            start=(m == 0), stop=(m == 3),
                        )
                    if nf == 0:
                        nc.vector.tensor_copy(o_sb[:, 0:512], pf[:])
                    else:
                        nc.scalar.copy(o_sb[:, 512:1024], pf[:])
                eng = nc.sync if t % 2 == 0 else nc.scalar
                eng.dma_start(out_d[t * P:(t + 1) * P, :], o_sb[:])

            def attention(pr, emit_out=False):
                pt_tiles = {}
                scores(pr, 0, pt_tiles)
                nwin = 2 if emit_out else 4
                for w0 in range(0, NW, nwin):
                    for t in range(w0 + 1, w0 + nwin + 1):
                        scores(pr, t, pt_tiles)
                    epilogue(pr, w0, nwin, pt_tiles)
                    if emit_out:
                        for t in range(w0, w0 + nwin):
                            out_window(t)

            # issue order: interleave projections with attention head-pairs so
            # the PE always has dense matmul work while ACT/DVE chew on the
            # softmax chain of the previous head-pair
            # early half (token cols < XSPLIT) unlocks as soon as the first
            # xT DMA lands; late half follows the second xT DMA
            proj_qk(0, [(128, 512), (640, 512)])
            proj_qk(4, [(0, 512), (512, 512)])
            proj_v(range(0, 9))
            proj_qk(0, [(1152, 512), (1664, 512)])
            proj_qk(4, [(1024, 512), (1536, 512), (2048, 128)])
            proj_v(range(9, NW + 1))
            attention(0)
            for pr in range(1, 3):
                proj_qk(pr)
                proj_qk(4 + pr)
                attention(pr)
            proj_qk(3)
            proj_qk(7)
            attention(3, emit_out=True)

    nc.compile()
    return nc


def _get_nc():
    if "nc" not in _CACHE:
        _CACHE["nc"] = _build()
    return _CACHE["nc"]


def kernel(x, w_qkv, w_out, b_out):
    x = np.asarray(x, dtype=np.float32)
    w_qkv_b = np.asarray(w_qkv, dtype=np.float32).astype(ml_dtypes.bfloat16)
    w_out_b = np.asarray(w_out, dtype=np.float32).astype(ml_dtypes.bfloat16)
    b_out = np.asarray(b_out, dtype=np.float32)

    # maskT[j, i] = 1 where j <= i  (transposed causal mask for current window)
    maskT = np.triu(np.ones((P, P), dtype=np.float32)).astype(ml_dtypes.bfloat16)
    ones1 = np.ones((1, DHEAD), dtype=ml_dtypes.bfloat16)

    x_pad = np.concatenate([np.zeros((WSZ, DIM), np.float32), x], axis=0)
    in_maps = []
    for c in range(NCORES):
        x_sh = x_pad[c * R:c * R + T]                       # (2176, 1024)
        xT = np.ascontiguousarray(x_sh.T).astype(ml_dtypes.bfloat16)
        in_maps.append({
            "xT": xT,
            "wqkv": w_qkv_b,
            "wout": w_out_b,
            "maskT": maskT,
            "ones1": ones1,
        })

    nc = _get_nc()
    res = run_bass_kernel_spmd(nc, in_maps, core_ids=list(range(NCORES)))
    out = np.concatenate(
        [np.asarray(res.results[c]["out"]).astype(np.float32) for c in range(NCORES)],
        axis=0,
    )
    return out + b_out[None, :]
